# revision 3
# baseline (speedup 1.0000x reference)
"""GNN message-passing layer on 8 Trainium2 NeuronCores.

Strategy: edges are bucketed by destination node (6250 nodes/core) so the
segment-sum is core-local (no collectives).  The host pre-gathers the
endpoint node features into edge-slot order (the same class of host prep
as the edge-feature scatter), so ALL device DMA is large and sequential —
no SWDGE gathers, no on-device A/B tables.

Host prep per core:
  - LPT bin-packing of the 6250 local dst nodes into 49 blocks of <=128
    nodes and <=21*128 edges; within a block, heavy nodes sit at central
    slots and each slot's edges are assigned to tiles whose 32-wide
    one-hot window covers the slot (windows slide across the block).
  - sd_t  [128, SLOTS] bf16: rows 0-63 nf[src], 64-127 nf[dst], slot-major
  - ef_t  [65, SLOTS]  bf16: rows 0-63 edge_feat, row 64 ones (bias row)
  - dstr  [128, TILES] bf16: dst slot relative to the tile's window (-1 pad)

Device per tile of 128 edges:
  z = sd_tile.T @ W_sd  (+)  ef_tile.T @ W_e65   (PSUM accumulate,
      contraction 128 + 65; bias via the ones row)
  h = SiLU(z)  on ACT, batched 21 tiles (one block) per PSUM region
  segment-sum: psH[:, win] += h_tile.T @ onehot32_tile, windows slide
      over the block's 128 slots (PSUM has_written handles first-touch),
      then one DVE copy into hall feature-major.
Phase 3 (update MLP + residual + LayerNorm) is feature-major per 512
columns.  DMA rings: sd on SP(HWDGE), ef on Pool(SWDGE), nfl/out split.

Output is written feature-major [64, 6272] per core; host inverse-permutes
slots -> nodes and concatenates.
"""
import sys
import bisect
sys.path.insert(0, "/opt/trn_rl_repo")
import numpy as np
import ml_dtypes

import concourse.bass as bass
import concourse.bacc as bacc
import concourse.mybir as mybir
import concourse.tile as tile
from concourse.bass_utils import run_bass_kernel_spmd

F32 = mybir.dt.float32
BF16 = mybir.dt.bfloat16

N_NODES = 50000
N_EDGES = 1000000
D = 64
NC = 8
NPC = 6250              # nodes per core
BLOCKS = 49             # 49 blocks of 128 dst slots
T = 21                  # tiles (of 128 edges) per block
EPB = T * 128           # edge slots per block (2688)
NLOC = BLOCKS * 128     # 6272 node slots
TILES = BLOCKS * T      # 1029
SLOTS = TILES * 128     # 131712 edge slots per core
WIN = 32                # one-hot window width
BASES = [(t * (128 - WIN)) // (T - 1) for t in range(T)]
CHUNKS = [2] * 24 + [1]  # blocks per phase-2 chunk
LN_EPS = 1e-5

_CACHED = {}


def _build_bass():
    nc = bacc.Bacc("TRN2", target_bir_lowering=False, debug=False,
                   num_devices=NC)

    # ---- I/O ----
    sd_t = nc.dram_tensor("sd_t", [128, SLOTS], BF16, kind="ExternalInput")
    ef_t = nc.dram_tensor("ef_t", [65, SLOTS], BF16, kind="ExternalInput")
    dstr = nc.dram_tensor("dstr", [128, TILES], BF16, kind="ExternalInput")
    deg = nc.dram_tensor("deg", [1, NLOC], F32, kind="ExternalInput")
    iota = nc.dram_tensor("iota", [128, 128], BF16, kind="ExternalInput")
    nf_loc = nc.dram_tensor("nf_loc", [D, NLOC], F32, kind="ExternalInput")
    w_sd = nc.dram_tensor("w_sd", [128, D], BF16, kind="ExternalInput")
    w_e65 = nc.dram_tensor("w_e65", [65, D], BF16, kind="ExternalInput")
    w_m2e = nc.dram_tensor("w_m2e", [65, D], F32, kind="ExternalInput")
    w_u1 = nc.dram_tensor("w_u1", [D, D], F32, kind="ExternalInput")
    b_u1 = nc.dram_tensor("b_u1", [D, 1], F32, kind="ExternalInput")
    w_u2e = nc.dram_tensor("w_u2e", [65, D], F32, kind="ExternalInput")
    gam = nc.dram_tensor("gam", [D, 1], F32, kind="ExternalInput")
    bet = nc.dram_tensor("bet", [D, 1], F32, kind="ExternalInput")
    consts = nc.dram_tensor("consts", [D, 2], F32, kind="ExternalInput")
    onesr = nc.dram_tensor("onesr", [1, 512], F32, kind="ExternalInput")
    out_fm = nc.dram_tensor("out_fm", [D, NLOC], F32, kind="ExternalOutput")

    with tile.TileContext(nc) as tc:
        with tc.tile_pool(name="persist", bufs=1) as pp:
            # persistent SBUF state
            dstr_sb = pp.tile([128, TILES], BF16, tag="dstr")
            iota_sb = pp.tile([128, 128], BF16, tag="iota")
            wsd_sb = pp.tile([128, D], BF16, tag="wsd")
            we_sb = pp.tile([65, D], BF16, tag="we")
            wm2_sb = pp.tile([65, D], F32, tag="wm2")
            wu1_sb = pp.tile([D, D], F32, tag="wu1")
            bu1_sb = pp.tile([D, 1], F32, tag="bu1")
            wu2_sb = pp.tile([65, D], F32, tag="wu2")
            gam_sb = pp.tile([D, 1], F32, tag="gam")
            bet_sb = pp.tile([D, 1], F32, tag="bet")
            hall = pp.tile([65, NLOC], F32, tag="hall")
            cst = pp.tile([D, 2], F32, tag="cst")
            onesr_sb = pp.tile([1, 512], F32, tag="onesr")
            oinv = cst[:, 0:1]     # 1/64 column
            eps_sb = cst[0:1, 1:2]
            ones1 = onesr_sb[:, 0:D]

            nc.sync.dma_start(dstr_sb[:], dstr[:])
            nc.sync.dma_start(iota_sb[:], iota[:])
            nc.sync.dma_start(wsd_sb[:], w_sd[:])
            nc.sync.dma_start(we_sb[:], w_e65[:])
            nc.sync.dma_start(wm2_sb[:], w_m2e[:])
            nc.sync.dma_start(wu1_sb[:], w_u1[:])
            nc.sync.dma_start(bu1_sb[:], b_u1[:])
            nc.sync.dma_start(wu2_sb[:], w_u2e[:])
            nc.sync.dma_start(gam_sb[:], gam[:])
            nc.sync.dma_start(bet_sb[:], bet[:])
            nc.sync.dma_start(hall[64:65, :], deg[:])
            nc.sync.dma_start(cst[:], consts[:])
            nc.sync.dma_start(onesr_sb[:], onesr[:])

            # ---------------- phase 2: edges ----------------
            NBM = max(CHUNKS)
            with (
                tc.tile_pool(name="p2", bufs=3) as p2,
                tc.tile_pool(name="p2h", bufs=3) as p2h,
                tc.tile_pool(name="p2ps", bufs=2, space="PSUM") as p2ps,
                tc.tile_pool(name="p2ph", bufs=2, space="PSUM") as p2ph,
            ):
                t0 = 0      # global tile base of chunk
                for ci, nb in enumerate(CHUNKS):
                    nt = nb * T
                    W = nt * 128
                    sd_c = p2.tile([128, NBM * EPB], BF16, tag="sd")
                    nc.sync.dma_start(sd_c[:, :W],
                                      sd_t[:, t0 * 128:(t0 + nt) * 128])
                    ef_c = p2.tile([65, NBM * EPB], BF16, tag="ef")
                    nc.gpsimd.dma_start(ef_c[:, :W],
                                        ef_t[:, t0 * 128:(t0 + nt) * 128])

                    # windowed one-hot [edge, win-slot] per chunk
                    oh = p2h.tile([128, NBM * T * WIN], BF16, tag="oh")
                    nc.vector.tensor_tensor(
                        out=oh[:, :nt * WIN].rearrange("p (t x) -> p t x", t=nt),
                        in0=iota_sb[:, None, 0:WIN]
                        .to_broadcast([128, nt, WIN]),
                        in1=dstr_sb[:, t0:t0 + nt, None]
                        .to_broadcast([128, nt, WIN]),
                        op=mybir.AluOpType.is_equal,
                    )

                    h = p2h.tile([128, NBM * T * D], BF16, tag="h")
                    for b in range(nb):
                        ps = p2ps.tile([128, T * D], F32, tag="ps")
                        for t in range(T):
                            tt = b * T + t
                            col = tt * 128
                            nc.tensor.matmul(
                                ps[:, t * D:(t + 1) * D],
                                lhsT=sd_c[:, col:col + 128],
                                rhs=wsd_sb[:], start=True, stop=False,
                                skip_group_check=True,
                            )
                            nc.tensor.matmul(
                                ps[:, t * D:(t + 1) * D],
                                lhsT=ef_c[:, col:col + 128],
                                rhs=we_sb[:], start=False, stop=True,
                                skip_group_check=True,
                            )
                        nc.scalar.activation(
                            h[:, b * T * D:(b + 1) * T * D], ps[:],
                            mybir.ActivationFunctionType.Silu)

                        # windowed segment-sum of the block's 21 tiles
                        psH = p2ph.tile([D, 128], F32, tag="psH")
                        for t in range(T):
                            tt = b * T + t
                            nc.tensor.matmul(
                                psH[:, BASES[t]:BASES[t] + WIN],
                                lhsT=h[:, tt * D:(tt + 1) * D],
                                rhs=oh[:, tt * WIN:(tt + 1) * WIN],
                                start=(t == 0), stop=(t == T - 1),
                                skip_group_check=True,
                            )
                        gblk = t0 // T + b
                        nc.vector.tensor_copy(
                            hall[0:D, gblk * 128:(gblk + 1) * 128], psH[:])
                    t0 += nt

            # ---------------- phase 3: update MLP + LayerNorm ------------
            with (
                tc.tile_pool(name="p3", bufs=2) as p3,
                tc.tile_pool(name="p3ps", bufs=1, space="PSUM") as p3ps,
            ):
                for cs in range(0, NLOC, 512):
                    w = min(512, NLOC - cs)
                    sl = slice(cs, cs + w)
                    ps_a = p3ps.tile([D, 512], F32, tag="ps_a")
                    nc.tensor.matmul(ps_a[:, :w], lhsT=wm2_sb[:], rhs=hall[:, sl],
                                     start=True, stop=True)
                    agg = p3.tile([D, 512], F32, tag="agg")
                    nc.vector.tensor_copy(agg[:, :w], ps_a[:, :w])

                    ps_u1 = p3ps.tile([D, 512], F32, tag="ps_u1")
                    nc.tensor.matmul(ps_u1[:, :w], lhsT=wu1_sb[:], rhs=agg[:, :w],
                                     start=True, stop=True)
                    s1 = p3.tile([65, 512], F32, tag="s1")
                    nc.vector.tensor_copy(s1[64:65, :w], onesr_sb[:, :w])
                    nc.scalar.activation(s1[0:D, :w], ps_u1[:, :w],
                                         mybir.ActivationFunctionType.Silu,
                                         bias=bu1_sb[:])
                    ps_u2 = p3ps.tile([D, 512], F32, tag="ps_u2")
                    nc.tensor.matmul(ps_u2[:, :w], lhsT=wu2_sb[:], rhs=s1[:, :w],
                                     start=True, stop=True)

                    nfl = p3.tile([D, 512], F32, tag="nfl")
                    nc.scalar.dma_start(nfl[:, :w], nf_loc[:, sl])
                    xr = p3.tile([D, 512], F32, tag="xr")
                    nc.vector.tensor_add(xr[:, :w], ps_u2[:, :w], nfl[:, :w])

                    sq = p3.tile([D, 512], F32, tag="sq")
                    nc.scalar.activation(sq[:, :w], xr[:, :w],
                                         mybir.ActivationFunctionType.Square)
                    ps_s1 = p3ps.tile([1, 512], F32, tag="ps_s1")
                    nc.tensor.matmul(ps_s1[:, :w], lhsT=oinv[:], rhs=xr[:, :w],
                                     start=True, stop=True)
                    ps_s2 = p3ps.tile([1, 512], F32, tag="ps_s2")
                    nc.tensor.matmul(ps_s2[:, :w], lhsT=oinv[:], rhs=sq[:, :w],
                                     start=True, stop=True)
                    mean_sb = p3.tile([1, 512], F32, tag="mean_sb")
                    nc.vector.tensor_copy(mean_sb[:, :w], ps_s1[:, :w])
                    msq = p3.tile([1, 512], F32, tag="msq")
                    nc.vector.tensor_mul(msq[:, :w], mean_sb[:, :w], mean_sb[:, :w])
                    var = p3.tile([1, 512], F32, tag="var")
                    nc.vector.tensor_tensor(out=var[:, :w], in0=ps_s2[:, :w],
                                            in1=msq[:, :w],
                                            op=mybir.AluOpType.subtract)
                    std = p3.tile([1, 512], F32, tag="std")
                    nc.scalar.activation(std[:, :w], var[:, :w],
                                         mybir.ActivationFunctionType.Sqrt,
                                         bias=eps_sb[:])
                    rstd = p3.tile([1, 512], F32, tag="rstd")
                    nc.vector.reciprocal(rstd[:, :w], std[:, :w])

                    ps_mb = p3ps.tile([D, 512], F32, tag="ps_mb")
                    nc.tensor.matmul(ps_mb[:, :w], lhsT=ones1[:], rhs=mean_sb[:, :w],
                                     start=True, stop=True)
                    ps_rb = p3ps.tile([D, 512], F32, tag="ps_rb")
                    nc.tensor.matmul(ps_rb[:, :w], lhsT=ones1[:], rhs=rstd[:, :w],
                                     start=True, stop=True)

                    t1_ = p3.tile([D, 512], F32, tag="t1")
                    nc.vector.tensor_tensor(out=t1_[:, :w], in0=xr[:, :w],
                                            in1=ps_mb[:, :w],
                                            op=mybir.AluOpType.subtract)
                    t2_ = p3.tile([D, 512], F32, tag="t2")
                    nc.vector.tensor_mul(t2_[:, :w], t1_[:, :w], ps_rb[:, :w])
                    oc = p3.tile([D, 512], F32, tag="oc")
                    nc.scalar.activation(oc[:, :w], t2_[:, :w],
                                         mybir.ActivationFunctionType.Identity,
                                         bias=bet_sb[:], scale=gam_sb[:])
                    nc.sync.dma_start(out_fm[:, sl], oc[:, :w])

    nc.compile()
    return nc


def _binpack(deg):
    """LPT: pack NPC dst nodes into BLOCKS bins, <=128 nodes and <=EPB
    edges per bin, balancing edge counts.  Returns list of node arrays."""
    order = np.argsort(-deg, kind="stable")
    binsum = np.zeros(BLOCKS, np.int64)
    bincnt = np.zeros(BLOCKS, np.int64)
    bins = [[] for _ in range(BLOCKS)]
    big = np.iinfo(np.int64).max
    for n in order:
        d = deg[n]
        feas = (bincnt < 128) & (binsum + d <= EPB)
        if not feas.any():
            raise ValueError("binpack infeasible")
        b = int(np.where(feas, binsum, big).argmin())
        bins[b].append(n)
        bincnt[b] += 1
        binsum[b] += d
    return bins


_CENTER_OUT = None


def _center_out():
    global _CENTER_OUT
    if _CENTER_OUT is None:
        seq = []
        lo, hi = 63, 64
        while len(seq) < 128:
            seq.append(hi); hi += 1
            if len(seq) < 128:
                seq.append(lo); lo -= 1
        _CENTER_OUT = np.array(seq)
    return _CENTER_OUT


def _prep(node_feat, edge_src, edge_dst, edge_feat,
          W_m1, b_m1, W_m2, b_m2, W_u1, b_u1, W_u2, b_u2,
          ln_gamma, ln_beta):
    """Host-side sharding: bucket edges by dst, bin-pack dst nodes into
    blocks, window-assign edges to tiles, pre-gather endpoint features."""
    bf = ml_dtypes.bfloat16
    order = np.argsort(edge_dst, kind="stable")
    sdst = edge_dst[order]
    nf_bf = node_feat.astype(bf)
    ef_bf = edge_feat.astype(bf)
    cseq = _center_out()

    w_sd = np.ascontiguousarray(W_m1[0:2 * D]).astype(bf)      # [128, 64]
    w_e65 = np.zeros((65, D), np.float32)
    w_e65[0:D] = W_m1[2 * D:3 * D]
    w_e65[64] = b_m1
    w_m2e = np.zeros((65, D), np.float32)
    w_m2e[0:D] = W_m2
    w_m2e[64] = b_m2
    w_u2e = np.zeros((65, D), np.float32)
    w_u2e[0:D] = W_u2
    w_u2e[64] = b_u2
    iota = np.tile(np.arange(128, dtype=np.float32), (128, 1))

    common = {
        "iota": iota.astype(bf),
        "w_sd": w_sd, "w_e65": w_e65.astype(bf),
        "w_m2e": w_m2e, "w_u1": np.ascontiguousarray(W_u1),
        "b_u1": b_u1.reshape(D, 1).astype(np.float32), "w_u2e": w_u2e,
        "gam": ln_gamma.reshape(D, 1).astype(np.float32),
        "bet": ln_beta.reshape(D, 1).astype(np.float32),
        "consts": np.stack([np.full(D, 1.0 / 64.0, np.float32),
                            np.full(D, LN_EPS, np.float32)], axis=1),
        "onesr": np.ones((1, 512), np.float32),
    }

    in_maps, perms = [], []
    for c in range(NC):
        lo, hi = c * NPC, (c + 1) * NPC
        e0, e1 = np.searchsorted(sdst, lo), np.searchsorted(sdst, hi)
        eidx = order[e0:e1]                         # sorted by local dst
        ldst = sdst[e0:e1] - lo
        deg = np.bincount(ldst, minlength=NPC)
        starts = np.searchsorted(ldst, np.arange(NPC + 1))

        bins = _binpack(deg)
        slot_of_node = np.empty(NPC, np.int64)
        pos_of_edge = np.empty(len(eidx), np.int64)
        dstr_e = np.full((TILES, 128), -1.0, np.float32)

        for b in range(BLOCKS):
            nodes = np.array(bins[b], dtype=np.int64)
            if len(nodes) == 0:
                continue
            degs = deg[nodes]
            o = np.argsort(-degs, kind="stable")
            sib = np.empty(len(nodes), np.int64)
            sib[o] = cseq[:len(nodes)]
            slot_of_node[nodes] = b * 128 + sib
            cap = np.full(T, 128, np.int64)
            fill = np.zeros(T, np.int64)
            for s, nd in sorted(zip(sib, nodes)):
                d = deg[nd]
                if d == 0:
                    continue
                t_min = bisect.bisect_right(BASES, s - WIN)
                t_max = bisect.bisect_right(BASES, s) - 1
                off = 0
                for t in range(t_min, t_max + 1):
                    if cap[t] == 0:
                        continue
                    take = min(cap[t], d - off)
                    if take <= 0:
                        break
                    gt = b * T + t
                    p0 = gt * 128 + fill[t]
                    pos_of_edge[starts[nd] + off:starts[nd] + off + take] = \
                        p0 + np.arange(take)
                    dstr_e[gt, fill[t]:fill[t] + take] = float(s - BASES[t])
                    cap[t] -= take
                    fill[t] += take
                    off += take
                assert off == d, f"window overflow: core {c} blk {b} slot {s}"

        sd = np.zeros((SLOTS, 128), bf)
        sd[pos_of_edge, 0:D] = nf_bf[edge_src[eidx]]
        sd[pos_of_edge, D:128] = nf_bf[edge_dst[eidx]]
        ef = np.zeros((SLOTS, 65), bf)
        ef[pos_of_edge, 0:D] = ef_bf[eidx]
        ef[:, 64] = 1.0

        degc = np.zeros((1, NLOC), np.float32)
        degc[0, slot_of_node] = deg
        nfl = np.zeros((D, NLOC), np.float32)
        nfl[:, slot_of_node] = node_feat[lo:hi].T

        in_maps.append({
            **common,
            "sd_t": np.ascontiguousarray(sd.T),
            "ef_t": np.ascontiguousarray(ef.T),
            "dstr": np.ascontiguousarray(dstr_e.T).astype(bf),
            "deg": degc,
            "nf_loc": nfl,
        })
        perms.append(slot_of_node)
    return in_maps, perms


def kernel(**inputs):
    inputs = {k: np.asarray(v) for k, v in inputs.items()}
    in_maps, perms = _prep(**inputs)
    if "nc" not in _CACHED:
        _CACHED["nc"] = _build_bass()
    res = run_bass_kernel_spmd(_CACHED["nc"], in_maps, list(range(NC)))
    out = np.empty((N_NODES, D), np.float32)
    for c in range(NC):
        out[c * NPC:(c + 1) * NPC] = res.results[c]["out_fm"].T[perms[c]]
    return out


if __name__ == "__main__":
    sys.path.insert(0, "/root/problem")
    import reference
    inputs = {k: np.asarray(v) for k, v in reference.setup_inputs().items()}
    exp = np.asarray(reference.reference(**inputs))
    got = kernel(**inputs)
    err = np.abs(got - exp).max() / (np.abs(exp).max() + 1e-30)
    print("Relative error:", err)
